# revision 13
# baseline (speedup 1.0000x reference)
"""Single-layer transformer LM head kernel for 8 Trainium2 NeuronCores.

Model (B=2, T=2048, D=1024, V=32000):
    x = tok_emb[idx] + pos_emb
    x = x + 0.125 * causal_attn(x@Wq, x@Wk, x@Wv)
    x = x + gelu(x@W1 + b1)@W2 + b2
    out = x@Wout + bout

v2 design -- fully collective-free (no cross-core communication at all):
  - trunk token-parallel: core c owns 512 tokens (batch c//4, block c%4).
    Every core recomputes K/V for its whole batch-sequence from
    block-ROTATED embeddings (own 512 tokens always first), with causality
    as a per-core additive mask input.
  - logits TOKEN-parallel: each core computes its own 512 tokens x the
    FULL 32000-dim vocab, streaming Wout (bf16, 64MB) under the logits
    compute. No final-hidden AllGather, no barrier, no stragglers.
  - logits computed vocab-on-partitions: out[vc,128,512] so the bout bias
    is a per-partition scalar fused into the psum->sbuf Copy activation,
    and output DMAs are fully contiguous 256KB blocks.
  - QKV projections + attention (scores, attnV) run in fp8(e4m3)
    DoubleRow mode (2 k-tiles per instruction, ~1.5x measured over bf16);
    K/V live in SBUF in fp8 (no DRAM round-trip). MLP + logits run in
    bf16 (1 row/cycle, same as f32r, half the DMA/SBUF of f32); fc1 also
    runs fp8-DR (the residual path stays bf16).
  - residual stream carried in bf16, pre-scaled by SX=2^9 (all scales are
    powers of two, exact; W1/Wout divided by SX and W2/b2 multiplied by
    SX on host so the carry scale cancels). Measured end-to-end numeric
    error of this scheme vs the f32 reference: 1.40e-2 (gate: 2e-2).
"""
import numpy as np
import ml_dtypes
import concourse.bass as bass
import concourse.bacc as bacc
import concourse.tile as tile
from concourse import bass_utils, mybir

F32 = mybir.dt.float32
BF16 = mybir.dt.bfloat16
F8 = mybir.dt.float8e4
AF = mybir.ActivationFunctionType
OP = mybir.AluOpType
DR = mybir.MatmulPerfMode.DoubleRow
E4M3 = ml_dtypes.float8_e4m3
NBF16 = ml_dtypes.bfloat16

N_CORES = 8
B, T, D, DH, V = 2, 2048, 1024, 4096, 32000
TB = T // 4            # 512 tokens per core
KC = D // 128          # 8 d_model chunks
KP = KC // 2           # 4 d_model chunk-pairs (DoubleRow)
HC = DH // 128         # 32 d_hidden chunks
NTK = T // 128         # 16 key chunks
NV = V // 128          # 250 vocab chunks

# power-of-2 scales (from the fixed input distribution; see v2_sim.py)
SX = 2.0 ** 9          # residual-stream carry scale
SWQ = SWK = SWV = 2.0 ** 9
SQE = 2.0 ** -8        # q psum -> fp8 extra scale
SKE = 2.0 ** -9
SVE = 2.0 ** -9
SEFF = (1.0 / 32.0) / (SX * SX * SWQ * SWK * SQE * SKE)   # = 2^-24
RS_CONST = 0.125 / (SWV * SVE)                            # = 0.125
SW1 = 2.0 ** 9         # W1 fp8 scale (fc1 in fp8-DR)
SGE = 1.0 / (SX * SW1)  # gelu pre-activation descale
MASKV = -1.0e9 / SEFF

_STATE = {}
SCHED_FIX = True


def _trunk(nc, tc, io, dp, x2T, sub="all"):
    """Embeddings -> QKV (fp8 DR) -> attention (fp8 DR) -> MLP (bf16)."""
    with tc.tile_pool(name="trunk", bufs=1) as pp:
        x8F = pp.tile([128, KC, T], F8)          # embeddings, fp8 (x*SX)
        x0s = pp.tile([128, KC, TB], BF16)       # own block, bf16 (x*SX)
        kT8 = pp.tile([128, KC, T], F8)          # K^T * SX*SWK*SKE
        vT8 = pp.tile([128, NTK, D], F8)         # V (token-major) *SX*SWV*SVE
        qT8 = pp.tile([128, KC, TB], F8)
        attnT8 = pp.tile([128, NTK, TB], F8)     # exp(scores), [key, query]
        x1T = pp.tile([128, KC, TB], BF16)
        x18 = pp.tile([128, KC, TB], F8)
        hT = pp.tile([128, HC, TB], BF16)
        rs_b = pp.tile([128, TB], F32)

        # ---- embeddings (pipelined per 512-block) + V/K projections ----
        with tc.tile_pool(name="wqkv", bufs=1) as wp:
            wq_s = wp.tile([128, KP, 2, D], F8)
            wk_s = wp.tile([128, KP, 2, D], F8)
            wv_s = wp.tile([128, KP, 2, D], F8)
            _wq = nc.scalar if SCHED_FIX else nc.sync
            _wq.dma_start(wv_s[:], io["wv8"].ap())
            _wq.dma_start(wk_s[:], io["wk8"].ap())
            _wq.dma_start(wq_s[:], io["wq8"].ap())

            with tc.tile_pool(name="emb", bufs=5) as ep, \
                 tc.tile_pool(name="ps_pj", bufs=8, space="PSUM") as pspj:
                for tb in range(4):
                    for k in range(KC):
                        tok_s = ep.tile([128, TB], BF16, name="tok_s")
                        pos_s = ep.tile([128, TB], BF16, name="pos_s")
                        nc.sync.dma_start(
                            tok_s[:], io["xt_tok"].ap()[k][:, bass.ts(tb, TB)])
                        _pq = nc.gpsimd if SCHED_FIX else nc.sync
                        _pq.dma_start(
                            pos_s[:], io["xt_pos"].ap()[k][:, bass.ts(tb, TB)])
                        if tb == 0:
                            # own block: keep the bf16 sum as x0s
                            nc.vector.tensor_tensor(
                                out=x0s[:, k, :], in0=tok_s[:], in1=pos_s[:],
                                op=OP.add)
                            nc.scalar.activation(
                                x8F[:, k, bass.ts(tb, TB)], x0s[:, k, :],
                                AF.Copy)
                        else:
                            xsum = ep.tile([128, TB], BF16, name="xsum")
                            nc.vector.tensor_tensor(
                                out=xsum[:], in0=tok_s[:], in1=pos_s[:],
                                op=OP.add)
                            nc.scalar.activation(
                                x8F[:, k, bass.ts(tb, TB)], xsum[:], AF.Copy)

                    # V projection for this block: out [128 tok, 256 dout]
                    for tc_ in range(4 * tb, 4 * tb + 4):
                        for dh in range(4):
                            ps = pspj.tile([128, 256], F32, name="ps_pj")
                            for kp in range(KP):
                                nc.tensor.matmul(
                                    ps[:],
                                    x8F[:, 2 * kp:2 * kp + 2,
                                        bass.ts(tc_, 128)],
                                    wv_s[:, kp, :, bass.ts(dh, 256)],
                                    start=(kp == 0), stop=(kp == KP - 1),
                                    perf_mode=DR)
                            nc.vector.tensor_scalar_mul(
                                vT8[:, tc_, bass.ts(dh, 256)], ps[:], SVE)
                    # K projection for this block: out [128 dout, 256 tok]
                    for m in range(KC):
                        for th in range(2):
                            ps = pspj.tile([128, 256], F32, name="ps_pj")
                            for kp in range(KP):
                                nc.tensor.matmul(
                                    ps[:],
                                    wk_s[:, kp, :, bass.ts(m, 128)],
                                    x8F[:, 2 * kp:2 * kp + 2,
                                        512 * tb + 256 * th:
                                        512 * tb + 256 * th + 256],
                                    start=(kp == 0), stop=(kp == KP - 1),
                                    perf_mode=DR)
                            nc.vector.tensor_scalar_mul(
                                kT8[:, m, 512 * tb + 256 * th:
                                    512 * tb + 256 * th + 256], ps[:], SKE)
                    if tb == 0:
                        # Q projection (own block only)
                        for m in range(KC):
                            for th in range(2):
                                ps = pspj.tile([128, 256], F32, name="ps_pj")
                                for kp in range(KP):
                                    nc.tensor.matmul(
                                        ps[:],
                                        wq_s[:, kp, :, bass.ts(m, 128)],
                                        x8F[:, 2 * kp:2 * kp + 2,
                                            bass.ts(th, 256)],
                                        start=(kp == 0), stop=(kp == KP - 1),
                                        perf_mode=DR)
                                nc.vector.tensor_scalar_mul(
                                    qT8[:, m, bass.ts(th, 256)], ps[:], SQE)

        if sub == "proj":
            nc.vector.tensor_copy(x2T[:], x0s[:])
            return
        # ---------- attention ----------
        with tc.tile_pool(name="attn", bufs=1) as ap_, \
             tc.tile_pool(name="mskp", bufs=8) as mskp, \
             tc.tile_pool(name="stmp", bufs=4) as stp, \
             tc.tile_pool(name="ps_sc", bufs=4, space="PSUM") as ps_sc, \
             tc.tile_pool(name="ps_l", bufs=1, space="PSUM") as ps_lp, \
             tc.tile_pool(name="ps_av", bufs=3, space="PSUM") as ps_av:
            ones_f = ap_.tile([128, 1], F32)
            nc.vector.memset(ones_f[:], 1.0)
            ones8 = ap_.tile([128, 1], F8)
            nc.vector.tensor_copy(ones8[:], ones_f[:])
            ps_l = ps_lp.tile([1, TB], F32)

            for tkc in range(NTK):
                msk = mskp.tile([128, TB], F32, name="msk")
                nc.gpsimd.dma_start(msk[:], io["mask"].ap()[tkc])
                for qh in range(2):
                    ps = ps_sc.tile([128, 256], F32, name="ps_s")
                    for a in range(KP):
                        nc.tensor.matmul(
                            ps[:],
                            kT8[:, 2 * a:2 * a + 2, bass.ts(tkc, 128)],
                            qT8[:, 2 * a:2 * a + 2, bass.ts(qh, 256)],
                            start=(a == 0), stop=(a == KP - 1),
                            perf_mode=DR)
                    st = stp.tile([128, 256], F32, name="st")
                    nc.vector.tensor_tensor(
                        out=st[:], in0=ps[:], in1=msk[:, bass.ts(qh, 256)],
                        op=OP.add)
                    nc.scalar.activation(
                        attnT8[:, tkc, bass.ts(qh, 256)], st[:], AF.Exp,
                        scale=SEFF)
                if not SCHED_FIX:
                    nc.tensor.matmul(ps_l[:], ones8[:], attnT8[:, tkc, :],
                                     start=(tkc == 0), stop=(tkc == NTK - 1))
            if SCHED_FIX:
                for tkc in range(NTK):
                    nc.tensor.matmul(ps_l[:], ones8[:], attnT8[:, tkc, :],
                                     start=(tkc == 0), stop=(tkc == NTK - 1))

            # rs = RS_CONST / l, broadcast to 128 partitions via DRAM
            rs_row = ap_.tile([1, TB], F32)
            nc.vector.reciprocal(rs_row[:], ps_l[:])
            rs_row2 = ap_.tile([1, TB], F32)
            nc.vector.tensor_scalar_mul(rs_row2[:], rs_row[:], RS_CONST)
            rs_dram = dp.tile([1, TB], F32, name="rs_dram")
            nc.sync.dma_start(rs_dram[:], rs_row2[:])
            nc.sync.dma_start(rs_b[:], rs_dram[:].partition_broadcast(128))

            # attn @ V: out [128 dout, 256 q]; x1 = x0s + rs * out
            for m in range(KC):
                for qh in range(2):
                    ps = ps_av.tile([128, 256], F32, name="ps_o")
                    for a in range(NTK // 2):
                        nc.tensor.matmul(
                            ps[:],
                            vT8[:, 2 * a:2 * a + 2, bass.ts(m, 128)],
                            attnT8[:, 2 * a:2 * a + 2, bass.ts(qh, 256)],
                            start=(a == 0), stop=(a == NTK // 2 - 1),
                            perf_mode=DR)
                    ot = stp.tile([128, 256], F32, name="ot")
                    nc.vector.tensor_tensor(
                        out=ot[:], in0=ps[:], in1=rs_b[:, bass.ts(qh, 256)],
                        op=OP.mult)
                    nc.vector.tensor_tensor(
                        out=x1T[:, m, bass.ts(qh, 256)], in0=ot[:],
                        in1=x0s[:, m, bass.ts(qh, 256)], op=OP.add)
                    nc.scalar.activation(
                        x18[:, m, bass.ts(qh, 256)],
                        x1T[:, m, bass.ts(qh, 256)], AF.Copy)

        if sub == "attn":
            nc.vector.tensor_copy(x2T[:], x1T[:])
            return
        # ---------- MLP (bf16) ----------
        with tc.tile_pool(name="mlpc", bufs=1) as mp, \
             tc.tile_pool(name="w1p", bufs=3) as w1p, \
             tc.tile_pool(name="w2p", bufs=3) as w2p, \
             tc.tile_pool(name="ps_h", bufs=4, space="PSUM") as ps_hp:
            b1_s = mp.tile([128, HC], F32)
            b2_s = mp.tile([128, KC], F32)
            nc.sync.dma_start(b1_s[:], io["b1t"].ap())
            nc.sync.dma_start(b2_s[:], io["b2t"].ap())
            for m in range(HC):
                w1t = w1p.tile([128, KP, 2, 128], F8, name="w1t")
                nc.sync.dma_start(w1t[:], io["w1b"].ap()[m])
                for qh in range(2):
                    ps = ps_hp.tile([128, 256], F32, name="ps_f1")
                    for kp in range(KP):
                        nc.tensor.matmul(
                            ps[:], w1t[:, kp],
                            x18[:, 2 * kp:2 * kp + 2, bass.ts(qh, 256)],
                            start=(kp == 0), stop=(kp == KP - 1),
                            perf_mode=DR)
                    nc.scalar.activation(hT[:, m, bass.ts(qh, 256)], ps[:],
                                         AF.Gelu, bias=b1_s[:, m:m + 1],
                                         scale=SGE)
            for m in range(KC):
                w2t = w2p.tile([128, HC, 128], BF16, name="w2t")
                nc.sync.dma_start(w2t[:], io["w2b"].ap()[m])
                ps = ps_hp.tile([128, TB], F32, name="ps_mlp")
                for k in range(HC):
                    nc.tensor.matmul(ps[:], w2t[:, k, :], hT[:, k, :],
                                     start=(k == 0), stop=(k == HC - 1))
                nc.vector.scalar_tensor_tensor(
                    out=x2T[:, m, :], in0=ps[:], scalar=b2_s[:, m:m + 1],
                    in1=x1T[:, m, :], op0=OP.add, op1=OP.add)


def _logits(nc, tc, io, x2T):
    """Own 512 tokens x full vocab; vocab chunks on partitions."""
    out_d = io["logits"]
    with tc.tile_pool(name="lg", bufs=1) as lp, \
         tc.tile_pool(name="wop", bufs=8) as wop, \
         tc.tile_pool(name="ogp", bufs=6) as ogp, \
         tc.tile_pool(name="ps_lg", bufs=8, space="PSUM") as ps_lg:
        bout_s = lp.tile([128, NV], F32)
        nc.sync.dma_start(bout_s[:], io["boutb"].ap())
        for vc in range(NV):
            wot = wop.tile([128, KC, 128], BF16, name="wot")
            _wq = nc.sync if vc % 2 == 0 else nc.scalar
            _wq.dma_start(wot[:], io["woutb"].ap()[vc])
            ps = ps_lg.tile([128, TB], F32, name="ps_g")
            for k in range(KC):
                nc.tensor.matmul(ps[:], wot[:, k, :], x2T[:, k, :],
                                 start=(k == 0), stop=(k == KC - 1))
            og = ogp.tile([128, TB], BF16, name="og")
            nc.scalar.activation(og[:], ps[:], AF.Identity,
                                 bias=bout_s[:, vc:vc + 1])
            nc.gpsimd.dma_start(out_d.ap()[vc], og[:])


def _build(repeat=1, phases="full"):
    nc = bacc.Bacc("TRN2", target_bir_lowering=False, debug=False,
                   num_devices=N_CORES)
    io = {}
    def inp(name, shape, dt):
        io[name] = nc.dram_tensor(name, shape, dt, kind="ExternalInput")
    inp("xt_tok", [KC, 128, T], BF16)
    inp("xt_pos", [KC, 128, T], BF16)
    inp("wq8", [128, KP, 2, D], F8)
    inp("wk8", [128, KP, 2, D], F8)
    inp("wv8", [128, KP, 2, D], F8)
    inp("mask", [NTK, 128, TB], F32)
    inp("w1b", [HC, 128, KP, 2, 128], F8)
    inp("b1t", [128, HC], F32)
    inp("w2b", [KC, 128, HC, 128], BF16)
    inp("b2t", [128, KC], F32)
    inp("woutb", [NV, 128, KC, 128], BF16)
    inp("boutb", [128, NV], F32)
    io["logits"] = nc.dram_tensor("logits", [NV, 128, TB], BF16,
                                  kind="ExternalOutput")

    with tile.TileContext(nc) as tc:
        with tc.tile_pool(name="dram", bufs=1, space="DRAM") as dp:
            for _ in range(repeat):
                with tc.tile_pool(name="x2", bufs=1) as x2p:
                    x2T = x2p.tile([128, KC, TB], BF16)
                    if phases == "logits":
                        nc.vector.memset(x2T[:], 0.01)
                    if phases in ("full", "trunk", "proj", "attn"):
                        sub = {"proj": "proj", "attn": "attn"}.get(
                            phases, "all")
                        _trunk(nc, tc, io, dp, x2T, sub=sub)
                    if phases in ("full", "logits"):
                        _logits(nc, tc, io, x2T)

    nc.compile()
    return nc


def _prep_shared(Wq, Wk, Wv, W1, b1, W2, b2, Wout, bout, pos_emb):
    f = np.float32
    sh = {}

    def pack_qkv(W, s):
        # [D, D] -> [128, KP, 2, D] with d_in = 128*(2*kp + t) + p
        return np.ascontiguousarray(
            (W * s).reshape(KP, 2, 128, D).transpose(2, 0, 1, 3)
        ).astype(E4M3)

    sh["wq8"] = pack_qkv(np.asarray(Wq, f), SWQ)
    sh["wk8"] = pack_qkv(np.asarray(Wk, f), SWK)
    sh["wv8"] = pack_qkv(np.asarray(Wv, f), SWV)
    sh["w1b"] = np.ascontiguousarray(
        (np.asarray(W1, f) * SW1).reshape(KP, 2, 128, HC, 128)
        .transpose(3, 2, 0, 1, 4)).astype(E4M3)
    sh["b1t"] = np.ascontiguousarray(
        np.asarray(b1, f).reshape(HC, 128).T)
    sh["w2b"] = np.ascontiguousarray(
        (np.asarray(W2, f) * SX).reshape(HC, 128, KC, 128)
        .transpose(2, 1, 0, 3)).astype(NBF16)
    sh["b2t"] = np.ascontiguousarray(
        (np.asarray(b2, f) * SX).reshape(KC, 128).T)
    sh["woutb"] = np.ascontiguousarray(
        (np.asarray(Wout, f) / SX).reshape(KC, 128, NV, 128)
        .transpose(2, 1, 0, 3)).astype(NBF16)
    sh["boutb"] = np.ascontiguousarray(
        np.asarray(bout, f).reshape(NV, 128).T)

    orders = [[(j + i) % 4 for i in range(4)] for j in range(4)]
    pos = np.asarray(pos_emb[:T], f) * SX
    pos_rot = []
    for j in range(4):
        pr = np.concatenate([pos[TB * br:TB * (br + 1)] for br in orders[j]])
        pos_rot.append(np.ascontiguousarray(
            pr.T.reshape(KC, 128, T)).astype(NBF16))
    masks = []
    rr = np.arange(128)[:, None]
    cc = np.arange(TB)[None, :]
    for j in range(4):
        m = np.empty((NTK, 128, TB), dtype=f)
        for tkc in range(NTK):
            gtk = TB * orders[j][tkc // 4] + 128 * (tkc % 4) + rr
            m[tkc] = np.where(gtk <= TB * j + cc, 0.0, MASKV)
        masks.append(m)
    return sh, orders, pos_rot, masks


def make_in_maps(idx, tok_emb, pos_emb, Wq, Wk, Wv, W1, b1, W2, b2,
                 Wout, bout):
    f = np.float32
    tok_emb = np.asarray(tok_emb, f)
    idx = np.asarray(idx)
    sh, orders, pos_rot, masks = _prep_shared(
        Wq, Wk, Wv, W1, b1, W2, b2, Wout, bout, np.asarray(pos_emb, f))

    tok_full = [tok_emb[np.asarray(idx[b], dtype=np.int64)] * SX
                for b in range(B)]
    in_maps = []
    for c in range(N_CORES):
        b, j = c // 4, c % 4
        tr = np.concatenate([tok_full[b][TB * br:TB * (br + 1)]
                             for br in orders[j]])
        m = dict(sh)
        m["xt_tok"] = np.ascontiguousarray(
            tr.T.reshape(KC, 128, T)).astype(NBF16)
        m["xt_pos"] = pos_rot[j]
        m["mask"] = masks[j]
        in_maps.append(m)
    return in_maps


def kernel(idx, tok_emb, pos_emb, Wq, Wk, Wv, W1, b1, W2, b2, Wout, bout):
    if "nc" not in _STATE:
        _STATE["nc"] = _build()
    nc = _STATE["nc"]

    in_maps = make_in_maps(idx, tok_emb, pos_emb, Wq, Wk, Wv, W1, b1, W2,
                           b2, Wout, bout)
    res = bass_utils.run_bass_kernel_spmd(nc, in_maps,
                                          core_ids=list(range(N_CORES)))
    _STATE["last_results"] = res

    out = np.empty((B, T, V), dtype=np.float32)
    for c in range(N_CORES):
        b, j = c // 4, c % 4
        lg = np.asarray(res.results[c]["logits"], np.float32)
        out[b, TB * j:TB * (j + 1), :] = (
            lg.transpose(2, 0, 1).reshape(TB, V))
    return out
